# revision 1
# baseline (speedup 1.0000x reference)
"""Self-contained Trainium2 Bass kernel for nn_MultiHeadAttention_68367289417808.

kernel(**inputs) takes FULL unsharded inputs (as in reference.setup_inputs())
and returns the FULL [4, 2048, 1024] output.

Sharding: 8 cores = (batch 4) x (query-half 2); no collectives needed.
Each core runs the full per-shard MHA in fp32r (e8m11) on the tensor engine:
  - host pre-transposes/pre-rounds activations and weights
  - projections V->K->Q (DMA-balanced order) with fused bias
  - attention in transposed-scores layout [sk, sq]; masking done entirely on
    the tensor engine: scores += -1e4 * (1-mask) via a scaled-identity matmul
    (masked exp underflows to 0), then the masked weights are restored to
    ~exp(-1e-6)=1 by accumulating [vE|vO]^T @ (1-mask) value corrections
    (one M=128 matmul per head pair) plus a shared column-sum for the
    denominators; the softmax denominator rides along as a ones column.
  - output projection + bias; per-core [1024, 1024] slices assembled on host.
"""
import time

import jax
import numpy as np
from jax.experimental.shard_map import shard_map
from jax.sharding import Mesh, PartitionSpec

import concourse.bass as bass
import concourse.bacc as bacc
import concourse.mybir as mybir
import concourse.tile as tile
from concourse import bass2jax
from concourse.bass import ts, ds

F32 = mybir.dt.float32
F32R = mybir.dt.float32r
U8 = mybir.dt.uint8
AF = mybir.ActivationFunctionType
MULT = mybir.AluOpType.mult
ADD = mybir.AluOpType.add

P = 128
SQ = 1024
SK = 2048
D = 1024
H = 16
DK = 64
HK = 1024
BIG = 1.0e4


def round_f32r(a: np.ndarray) -> np.ndarray:
    """Round fp32 to fp32r (e8m11: keep 11 mantissa bits, RNE)."""
    a = np.ascontiguousarray(a, dtype=np.float32)
    u = a.view(np.uint32)
    add = np.uint32(0x7FF) + ((u >> np.uint32(12)) & np.uint32(1))
    u2 = (u + add) & np.uint32(0xFFFFF000)
    return u2.view(np.float32)


def build_mha():
    nc = bacc.Bacc("TRN2", target_bir_lowering=False)

    qT = nc.dram_tensor("qT", [D, SQ], F32R, kind="ExternalInput")
    kT = nc.dram_tensor("kT", [D, SK], F32R, kind="ExternalInput")
    vT = nc.dram_tensor("vT", [D, SK], F32R, kind="ExternalInput")
    mcT = nc.dram_tensor("mcT", [SK, SQ], U8, kind="ExternalInput")
    wq = nc.dram_tensor("wq", [D, HK], F32R, kind="ExternalInput")
    wk = nc.dram_tensor("wk", [D, HK], F32R, kind="ExternalInput")
    wv = nc.dram_tensor("wv", [D, HK], F32R, kind="ExternalInput")
    wo = nc.dram_tensor("wo", [HK, D], F32R, kind="ExternalInput")
    bq2 = nc.dram_tensor("bq2", [P, 8], F32, kind="ExternalInput")
    bk2 = nc.dram_tensor("bk2", [P, 8], F32, kind="ExternalInput")
    bvr = nc.dram_tensor("bvr", [1, HK], F32R, kind="ExternalInput")
    bor = nc.dram_tensor("bor", [1, D], F32R, kind="ExternalInput")
    onesd = nc.dram_tensor("onesd", [P, P], F32R, kind="ExternalInput")
    nident = nc.dram_tensor("nident", [P, P], F32R, kind="ExternalInput")
    out = nc.dram_tensor("out", [SQ, D], F32, kind="ExternalOutput")

    khT_d = nc.dram_tensor("khT_scr", [HK, SK], F32R)
    qhT_d = nc.dram_tensor("qhT_scr", [HK, SQ], F32R)
    vaug_d = nc.dram_tensor("vaug_scr", [SK, 8 * 130], F32R)
    rden_d = nc.dram_tensor("rden_scr", [H, SQ], F32)

    with tile.TileContext(nc) as tc:
        with tc.tile_pool(name="consts", bufs=1) as cst:
            ones_sb = cst.tile([P, P], F32R, tag="ones")
            nid_sb = cst.tile([P, P], F32R, tag="nid")
            bq_sb = cst.tile([P, 8], F32, tag="bq")
            bk_sb = cst.tile([P, 8], F32, tag="bk")
            bv_sb = cst.tile([1, HK], F32R, tag="bv")
            bo_sb = cst.tile([1, D], F32R, tag="bo")
            onecol = cst.tile([P, 16, 1], F32, tag="onecol")

            # ---------------- projections ----------------
            with (
                tc.tile_pool(name="wp", bufs=2) as wp,
                tc.tile_pool(name="xq", bufs=3) as xq,
                tc.tile_pool(name="pp", bufs=8, space="PSUM") as pp,
                tc.tile_pool(name="po", bufs=4) as po,
            ):
                # V projection -> vaug_d (startup phase)
                wv_sb = wp.tile([P, 8, HK], F32R, tag="w")
                vq0 = xq.tile([P, 8, 512], F32R, tag="xq")
                for _j in range(8):
                    nc.sync.dma_start(
                        vq0[:, _j],
                        vT.ap().rearrange("(j p) s -> p j s", p=P)[:, _j, ts(0, 512)],
                    )
                    nc.sync.dma_start(
                        wv_sb[:, _j],
                        wv.ap().rearrange("(j p) m -> p j m", p=P)[:, _j],
                    )
                nc.sync.dma_start(bq_sb[:], bq2.ap())
                nc.sync.dma_start(bk_sb[:], bk2.ap())
                nc.sync.dma_start(bv_sb[:], bvr.ap())
                nc.sync.dma_start(bo_sb[:], bor.ap())
                nc.sync.dma_start(ones_sb[:], onesd.ap())
                nc.sync.dma_start(nid_sb[:], nident.ap())
                nc.gpsimd.memset(onecol[:], 1.0)
                for c in range(4):
                    if c == 0:
                        vq = vq0
                    else:
                        vq = xq.tile([P, 8, 512], F32R, tag="xq")
                        for _j in range(8):
                            nc.sync.dma_start(
                                vq[:, _j],
                                vT.ap().rearrange("(j p) s -> p j s", p=P)[
                                    :, _j, ts(c, 512)
                                ],
                            )
                    vas = []
                    for blk in range(2):
                        # units u = (t_in, c2) pairs: blk0: t_in 0,1; blk1: 2,3
                        units = [(2 * blk + dt_, c2) for dt_ in range(2) for c2 in range(2)]
                        psvs = [pp.tile([P, 512], F32, tag="proj", name=f"psv{_u}") for _u in range(len(units))]
                        for j in range(8):
                            for u, (t_in, c2) in enumerate(units):
                                nc.tensor.matmul(
                                    psvs[u][:],
                                    vq[:, j, ts(t_in, P)],
                                    wv_sb[:, j, ts(c2, 512)],
                                    start=(j == 0),
                                    stop=False,
                                )
                        for u, (t_in, c2) in enumerate(units):
                            nc.tensor.matmul(
                                psvs[u][:],
                                ones_sb[0:1, 0:P],
                                bv_sb[:, ts(c2, 512)],
                                start=False,
                                stop=True,
                            )
                        for u, (t_in, c2) in enumerate(units):
                            if c2 == 0:
                                va = po.tile([P, 8, 130], F32R, tag="vaug")
                                vas.append((t_in, va))
                            else:
                                va = dict(vas)[t_in]
                            vag = va[:]
                            psv4 = psvs[u][:].rearrange(
                                "p (g h k) -> p g h k", g=4, h=2
                            )
                            nc.vector.tensor_copy(
                                vag[:, ds(4 * c2, 4), 0:130]
                                .rearrange("p g (h k) -> p g h k", h=2)[:, :, :, 0:64],
                                psv4,
                            )
                            if c2 == 1:
                                t = 4 * c + t_in
                                nc.vector.tensor_copy(
                                    vag[:, :, 64:130:65],
                                    onecol[:, :, 0].rearrange(
                                        "p (g h) -> p g h", h=2
                                    ),
                                )
                                nc.sync.dma_start(vaug_d.ap()[ts(t, P), :], va[:])
                        vas = [x for x in vas if x[0] >= 2 * blk + 2]

                # K projection -> khT_d
                wk_sb = wp.tile([P, 8, HK], F32R, tag="w")
                for _j in range(8):
                    nc.sync.dma_start(
                        wk_sb[:, _j],
                        wk.ap().rearrange("(j p) m -> p j m", p=P)[:, _j],
                    )
                for c in range(4):
                    kq = xq.tile([P, 8, 512], F32R, tag="xq")
                    for _j in range(8):
                        nc.sync.dma_start(
                            kq[:, _j],
                            kT.ap().rearrange("(j p) s -> p j s", p=P)[
                                :, _j, ts(c, 512)
                            ],
                        )
                    for iblk in range(2):
                        psks = [pp.tile([P, 512], F32, tag="proj", name=f"psk{_u}") for _u in range(4)]
                        for j in range(8):
                            for u in range(4):
                                nc.tensor.matmul(
                                    psks[u][:],
                                    wk_sb[:, j, ts(4 * iblk + u, P)],
                                    kq[:, j, :],
                                    start=(j == 0),
                                    stop=(j == 7),
                                )
                        for u in range(4):
                            i = 4 * iblk + u
                            ok = po.tile([P, 512], F32R, tag="projout")
                            nc.vector.tensor_scalar_add(
                                ok[:], psks[u][:], bk_sb[:, i : i + 1]
                            )
                            nc.sync.dma_start(khT_d.ap()[ts(i, P), ts(c, 512)], ok[:])

                # Q projection -> qhT_d
                wq_sb = wp.tile([P, 8, HK], F32R, tag="w")
                for _j in range(8):
                    nc.sync.dma_start(
                        wq_sb[:, _j],
                        wq.ap().rearrange("(j p) m -> p j m", p=P)[:, _j],
                    )
                for c in range(2):
                    qq = xq.tile([P, 8, 512], F32R, tag="xq")
                    for _j in range(8):
                        nc.sync.dma_start(
                            qq[:, _j],
                            qT.ap().rearrange("(j p) s -> p j s", p=P)[
                                :, _j, ts(c, 512)
                            ],
                        )
                    for iblk in range(2):
                        psqs = [pp.tile([P, 512], F32, tag="proj", name=f"psq{_u}") for _u in range(4)]
                        for j in range(8):
                            for u in range(4):
                                nc.tensor.matmul(
                                    psqs[u][:],
                                    wq_sb[:, j, ts(4 * iblk + u, P)],
                                    qq[:, j, :],
                                    start=(j == 0),
                                    stop=(j == 7),
                                )
                        for u in range(4):
                            i = 4 * iblk + u
                            oq = po.tile([P, 512], F32R, tag="projout")
                            nc.vector.tensor_scalar_add(
                                oq[:], psqs[u][:], bq_sb[:, i : i + 1]
                            )
                            nc.sync.dma_start(qhT_d.ap()[ts(i, P), ts(c, 512)], oq[:])

            # ---------------- attention ----------------
            with tc.tile_pool(name="aop", bufs=1) as aop:
              ao_sb = aop.tile([P, 8, SQ], F32R, tag="ao")
              with (
                tc.tile_pool(name="mcp", bufs=1) as mcp,
                  tc.tile_pool(name="khp", bufs=2) as khp,
                  tc.tile_pool(name="vap", bufs=2) as vap,
                  tc.tile_pool(name="qhp", bufs=2) as qhp,
                  tc.tile_pool(name="ep", bufs=2) as ep,
                  tc.tile_pool(name="mup", bufs=1) as mup,
                  tc.tile_pool(name="pss", bufs=2, space="PSUM") as pss,
                  tc.tile_pool(name="pso", bufs=1, space="PSUM") as pso,
                  tc.tile_pool(name="pcvp", bufs=1, space="PSUM") as pcvp,
                  tc.tile_pool(name="osb", bufs=2) as osb,
                  tc.tile_pool(name="cmp", bufs=1) as cmp,
                  tc.tile_pool(name="pcvs", bufs=1) as pcvs,
              ):
                  mc_sb = mcp.tile([P, 16, SQ], F32R, tag="mc")
                  for half in range(2):
                      mcu = mup.tile([P, 8, SQ], U8, tag="mcu")
                      nc.sync.dma_start(
                          mcu[:],
                          mcT.ap().rearrange("(t p) s -> p t s", p=P)[
                              :, ds(8 * half, 8), :
                          ],
                      )
                      for qtr in range(2):
                          nc.vector.tensor_copy(
                              mc_sb[:, ds(8 * half + 4 * qtr, 4), :],
                              mcu[:, ds(4 * qtr, 4), :],
                          )

                  # cmsum[sq] = sum_sk mc (shared denominator correction),
                  # broadcast to partition 64 via a ones K=1 matmul
                  ps_cm = pss.tile([1, SQ], F32, tag="pss")
                  for t in range(16):
                      for c2 in range(2):
                          nc.tensor.matmul(
                              ps_cm[:, ts(c2, 512)],
                              ones_sb[:, 0:1],
                              mc_sb[:, t, ts(c2, 512)],
                              start=(t == 0),
                              stop=(t == 15),
                          )
                  cmr = cmp.tile([1, SQ], F32R, tag="cm")
                  nc.vector.tensor_copy(cmr[:], ps_cm[:])
                  ps_cmb = pss.tile([65, SQ], F32, tag="pss")
                  for c2 in range(2):
                      nc.tensor.matmul(
                          ps_cmb[:, ts(c2, 512)],
                          ones_sb[0:1, 0:65],
                          cmr[:, ts(c2, 512)],
                          start=True,
                          stop=True,
                      )
                  cm_sb = cmp.tile([65, SQ], F32, tag="cm")
                  nc.vector.tensor_copy(cm_sb[:], ps_cmb[:])

                  for g in range(8):
                      khT_pair = khp.tile([P, SK], F32R, tag="kh")
                      nc.sync.dma_start(khT_pair[:], khT_d.ap()[ds(g * P, P), :])
                      vaug_pair = vap.tile([P, 16, 258], F32R, tag="va")
                      nc.sync.dma_start(
                          vaug_pair[:, :, 0:130],
                          vaug_d.ap().rearrange("(t p) c -> p t c", p=P)[
                              :, :, ds(g * 130, 130)
                          ],
                      )
                      for _hh in range(2):
                          nc.sync.dma_start(
                              vaug_pair[:, :, ds(130 + 64 * _hh, 64)],
                              vaug_d.ap().rearrange("(t p) c -> p t c", p=P)[
                                  :, :, ds(g * 130 + 65 * _hh, 64)
                              ],
                          )
                      qh_pair = qhp.tile([P, SQ], F32R, tag="qh")
                      nc.sync.dma_start(qh_pair[:], qhT_d.ap()[ds(g * P, P), :])
                      pcv = pcvp.tile([P, SQ], F32, tag="pcv")
                      pcv_sb = pcvs.tile([P, SQ], F32, tag="pcvsb")
                      for hh in range(2):
                          base = 64 * hh
                          ps_o = pso.tile([65, SQ], F32, tag="pso")
                          for t in range(16):
                              ps_s = pss.tile([P, SQ], F32, tag="pss")
                              for c2 in range(2):
                                  nc.tensor.matmul(
                                      ps_s[:, ts(c2, 512)],
                                      khT_pair[base : base + 64, ts(t, P)],
                                      qh_pair[base : base + 64, ts(c2, 512)],
                                      start=True,
                                      stop=False,
                                  )
                                  nc.tensor.matmul(
                                      ps_s[:, ts(c2, 512)],
                                      nid_sb[:],
                                      mc_sb[:, t, ts(c2, 512)],
                                      start=False,
                                      stop=True,
                                  )
                              e = ep.tile([P, SQ], F32R, tag="e")
                              nc.scalar.activation(e[:], ps_s[:], AF.Exp, scale=0.125)
                              for c2 in range(2):
                                  nc.tensor.matmul(
                                      ps_o[:, ts(c2, 512)],
                                      vaug_pair[:, t, ds(65 * hh, 65)],
                                      e[:, ts(c2, 512)],
                                      start=(t == 0),
                                      stop=False,
                                  )
                                  if hh == 0:
                                      # both heads' value corrections in one
                                      # M=128 matmul: rows 0:64 = head 2g,
                                      # rows 64:128 = head 2g+1
                                      nc.tensor.matmul(
                                          pcv[:, ts(c2, 512)],
                                          vaug_pair[:, t, 130:258],
                                          mc_sb[:, t, ts(c2, 512)],
                                          start=(t == 0),
                                          stop=(t == 15),
                                      )
                          if hh == 0:
                              nc.vector.tensor_copy(pcv_sb[:], pcv[:])
                          # normalize head 2g+hh
                          o_sb = osb.tile([65, SQ], F32, tag="osb")
                          nc.vector.tensor_tensor(
                              o_sb[0:64, :],
                              ps_o[0:64, :],
                              pcv_sb[base : base + 64, :],
                              ADD,
                          )
                          nc.vector.tensor_tensor(
                              o_sb[64:65, :], ps_o[64:65, :], cm_sb[64:65, :], ADD
                          )
                          nc.vector.reciprocal(o_sb[64:65, :], o_sb[64:65, :])
                          nc.sync.dma_start(rden_d.ap()[2 * g + hh : 2 * g + hh + 1, :], o_sb[64:65, :])
                          rbc = osb.tile([64, SQ], F32, tag="rbc")
                          nc.sync.dma_start(
                              rbc[:],
                              rden_d.ap()[2 * g + hh : 2 * g + hh + 1, :].to_broadcast((64, SQ)),
                          )
                          if hh == 0:
                              nc.vector.tensor_tensor(
                                  ao_sb[0:64, g, :], o_sb[0:64, :], rbc[:], MULT
                              )
                          else:
                              tmpn = osb.tile([64, SQ], F32R, tag="rbc")
                              nc.vector.tensor_tensor(
                                  tmpn[:], o_sb[0:64, :], rbc[:], MULT
                              )
                              nc.sync.dma_start(ao_sb[64:128, g, :], tmpn[:])

              # ---------------- output projection ----------------
              with (
                  tc.tile_pool(name="wop", bufs=1) as wop,
                  tc.tile_pool(name="pp2", bufs=8, space="PSUM") as pp2,
                  tc.tile_pool(name="po2", bufs=3) as po2,
              ):
                  wo_sb = wop.tile([P, 8, D], F32R, tag="wo")
                  for _j in range(8):
                      nc.sync.dma_start(
                          wo_sb[:, _j],
                          wo.ap().rearrange("(j p) m -> p j m", p=P)[:, _j],
                      )
                  scs = [(s, c) for s in range(8) for c in range(2)]
                  for blk in range(4):
                      units = scs[4 * blk : 4 * blk + 4]
                      psos = [pp2.tile([P, 512], F32, tag="op", name=f"pso2_{_u}") for _u in range(len(units))]
                      for j in range(8):
                          for u, (s, c) in enumerate(units):
                              nc.tensor.matmul(
                                  psos[u][:],
                                  ao_sb[:, j, ts(s, P)],
                                  wo_sb[:, j, ts(c, 512)],
                                  start=(j == 0),
                                  stop=False,
                              )
                      for u, (s, c) in enumerate(units):
                          nc.tensor.matmul(
                              psos[u][:],
                              ones_sb[0:1, 0:P],
                              bo_sb[:, ts(c, 512)],
                              start=False,
                              stop=True,
                          )
                          oo = po2.tile([P, 512], F32, tag="oo")
                          nc.vector.tensor_copy(oo[:], psos[u][:])
                          nc.sync.dma_start(out.ap()[ts(s, P), ts(c, 512)], oo[:])

    nc.compile()
    return nc


def make_host_inputs(q, k, v, mask, Wq, bq, Wk, bk, Wv, bv, Wo, bo):
    """Full inputs -> list of 8 per-core input dicts."""
    q = np.asarray(q, np.float32)
    k = np.asarray(k, np.float32)
    v = np.asarray(v, np.float32)
    mask = np.asarray(mask)
    r = round_f32r

    shared = {
        "wq": r(np.asarray(Wq, np.float32).transpose(1, 0, 2).reshape(D, HK)),
        "wk": r(np.asarray(Wk, np.float32).transpose(1, 0, 2).reshape(D, HK)),
        "wv": r(np.asarray(Wv, np.float32).transpose(1, 0, 2).reshape(D, HK)),
        "wo": r(np.asarray(Wo, np.float32)),
        "bq2": np.ascontiguousarray(
            np.asarray(bq, np.float32).reshape(HK).reshape(8, P).T
        ),
        "bk2": np.ascontiguousarray(
            np.asarray(bk, np.float32).reshape(HK).reshape(8, P).T
        ),
        "bvr": r(np.asarray(bv, np.float32).reshape(1, HK)),
        "bor": r(np.asarray(bo, np.float32).reshape(1, D)),
        "onesd": np.ones((P, P), np.float32),
        "nident": np.ascontiguousarray(-BIG * np.eye(P, dtype=np.float32)),
    }

    in_maps = []
    for core in range(8):
        b, j = divmod(core, 2)
        qs = q[b, j * SQ : (j + 1) * SQ, :]
        ms = mask[b, j * SQ : (j + 1) * SQ, :]
        m = dict(shared)
        m["qT"] = r(np.ascontiguousarray(qs.T))
        m["kT"] = r(np.ascontiguousarray(k[b].T))
        m["vT"] = r(np.ascontiguousarray(v[b].T))
        m["mcT"] = np.ascontiguousarray((~ms).T).astype(np.uint8)
        in_maps.append(m)
    return in_maps


def assemble_output(results):
    """8 per-core out [SQ, D] -> full [4, 2048, 1024]."""
    B, S = 4, 2048
    full = np.empty((B, S, D), np.float32)
    for core, res in enumerate(results):
        b, j = divmod(core, 2)
        full[b, j * SQ : (j + 1) * SQ, :] = res["out"]
    return full


class CompiledSpmd:
    def __init__(self, nc: bass.Bass, n_cores: int):
        bass2jax.install_neuronx_cc_hook()
        assert nc.dbg_addr is None, "build with debug=False"
        partition_name = (
            nc.partition_id_tensor.name if nc.partition_id_tensor else None
        )
        in_names, out_names, out_avals, zero_outs = [], [], [], []
        for alloc in nc.m.functions[0].allocations:
            if not isinstance(alloc, mybir.MemoryLocationSet):
                continue
            name = alloc.memorylocations[0].name
            if alloc.kind == "ExternalInput":
                if name != partition_name:
                    in_names.append(name)
            elif alloc.kind == "ExternalOutput":
                shape = tuple(alloc.tensor_shape)
                dtype = mybir.dt.np(alloc.dtype)
                out_names.append(name)
                out_avals.append(jax.core.ShapedArray(shape, dtype))
                zero_outs.append(np.zeros(shape, dtype))
        n_params = len(in_names)
        n_outs = len(out_avals)
        all_in_names = list(in_names) + list(out_names)
        if partition_name is not None:
            all_in_names.append(partition_name)

        def _body(*args):
            operands = list(args)
            if partition_name is not None:
                operands.append(bass2jax.partition_id_tensor())
            outs = bass2jax._bass_exec_p.bind(
                *operands,
                out_avals=tuple(out_avals),
                in_names=tuple(all_in_names),
                out_names=tuple(out_names),
                lowering_input_output_aliases=(),
                sim_require_finite=True,
                sim_require_nnan=True,
                nc=nc,
            )
            return tuple(outs)

        devices = jax.devices()[:n_cores]
        assert len(devices) == n_cores
        mesh = Mesh(np.asarray(devices), ("core",))
        self._mesh = mesh
        donate = tuple(range(n_params, n_params + n_outs))
        self._sharded = jax.jit(
            shard_map(
                _body,
                mesh=mesh,
                in_specs=(PartitionSpec("core"),) * (n_params + n_outs),
                out_specs=(PartitionSpec("core"),) * n_outs,
                check_rep=False,
            ),
            donate_argnums=donate,
            keep_unused=True,
        )
        self.in_names = in_names
        self.out_names = out_names
        self.out_avals = out_avals
        self.zero_outs = zero_outs
        self.n_cores = n_cores

    def _concat_inputs(self, in_maps):
        per_core = [[np.asarray(m[n]) for n in self.in_names] for m in in_maps]
        return [
            np.concatenate([per_core[c][i] for c in range(self.n_cores)], axis=0)
            for i in range(len(self.in_names))
        ]

    def run(self, in_maps, repeats: int = 1):
        """Returns (results_per_core, wall_times_s list of len repeats).

        Inputs and donated zero-output buffers are device_put outside the
        timed region, so wall time ~= dispatch + NEFF execution.
        """
        from jax.sharding import NamedSharding

        mesh = self._mesh
        shard = NamedSharding(mesh, PartitionSpec("core"))
        concat_in = [
            jax.device_put(a, shard) for a in self._concat_inputs(in_maps)
        ]
        rep_zeros = [
            [
                jax.device_put(
                    np.zeros((self.n_cores * z.shape[0], *z.shape[1:]), z.dtype),
                    shard,
                )
                for z in self.zero_outs
            ]
            for _ in range(repeats)
        ]
        jax.block_until_ready(concat_in)
        jax.block_until_ready(rep_zeros)
        times = []
        out_arrs = None
        for r in range(repeats):
            t0 = time.perf_counter()
            out_arrs = self._sharded(*concat_in, *rep_zeros[r])
            jax.block_until_ready(out_arrs)
            times.append(time.perf_counter() - t0)
        results = [
            {
                name: np.asarray(out_arrs[i]).reshape(
                    self.n_cores, *self.out_avals[i].shape
                )[c]
                for i, name in enumerate(self.out_names)
            }
            for c in range(self.n_cores)
        ]
        return results, times


_COMPILED = None


def _get_compiled():
    global _COMPILED
    if _COMPILED is None:
        nc = build_mha()
        _COMPILED = CompiledSpmd(nc, 8)
    return _COMPILED


def kernel(**inputs) -> np.ndarray:
    comp = _get_compiled()
    in_maps = make_host_inputs(**inputs)
    results, _ = comp.run(in_maps, repeats=1)
    return assemble_output(results)



# revision 5
# speedup vs baseline: 1.1768x; 1.1768x over previous
"""Self-contained Trainium2 Bass kernel for nn_MultiHeadAttention_68367289417808.

kernel(**inputs) takes FULL unsharded inputs (as in reference.setup_inputs())
and returns the FULL [4, 2048, 1024] output.

Sharding: 8 cores = (batch 4) x (query-half 2); no collectives needed.

Per-core pipeline (mixed precision, tuned against the TRN2 cost model):
  - Q/K projections in fp8e4m3 DoubleRow (weights x32, outputs stored as
    8*qh in fp8), V projection in fp8e4m3 DoubleRow with vh stored bf16.
  - scores = kh^T qh per head via fp8 DoubleRow (dk=64 split into two
    32-row k-tiles living on the same 32 partitions).
  - exp on the Act engine straight out of PSUM into bf16 (scale 2^-9
    compensates the 8x8 operand scaling); masked positions are then
    overwritten with exactly 1.0 (= exp(-1e-6) to ulp) via one DVE
    copy_predicated against a ones tile, matching the reference's
    masked_fill(-1e-6) semantics with no correction terms.
  - AV + softmax denominator (ones column in vaug) in bf16 matmuls,
    normalize on DVE (reciprocal + scalar_tensor_tensor), output
    projection + bias in bf16 with the final copy on the Act engine.
"""
import time

import jax
import ml_dtypes
import numpy as np
from jax.experimental.shard_map import shard_map
from jax.sharding import Mesh, PartitionSpec

import concourse.bass as bass
import concourse.bacc as bacc
import concourse.mybir as mybir
import concourse.tile as tile
from concourse import bass2jax
from concourse.bass import ts, ds

F32 = mybir.dt.float32
F8 = mybir.dt.float8e4
BF = mybir.dt.bfloat16
U16 = mybir.dt.uint16
AF = mybir.ActivationFunctionType
PM = mybir.MatmulPerfMode
MULT = mybir.AluOpType.mult
ADD = mybir.AluOpType.add

NF8 = ml_dtypes.float8_e4m3
NBF = ml_dtypes.bfloat16

P = 128
SQ = 1024
SK = 2048
D = 1024
H = 16
DK = 64
HK = 1024


def build_mha():
    nc = bacc.Bacc("TRN2", target_bir_lowering=False)

    qT8 = nc.dram_tensor("qT8", [D, SQ], F8, kind="ExternalInput")
    kT8 = nc.dram_tensor("kT8", [D, SK], F8, kind="ExternalInput")
    vT16 = nc.dram_tensor("vT16", [D, SK], BF, kind="ExternalInput")
    mskT = nc.dram_tensor("mskT", [SK, SQ], U16, kind="ExternalInput")
    wq8 = nc.dram_tensor("wq8", [D, HK], F8, kind="ExternalInput")
    wk8 = nc.dram_tensor("wk8", [D, HK], F8, kind="ExternalInput")
    wv16 = nc.dram_tensor("wv16", [D, HK], BF, kind="ExternalInput")
    wo16 = nc.dram_tensor("wo16", [HK, D], BF, kind="ExternalInput")
    bqc = nc.dram_tensor("bqc", [P, 8], F32, kind="ExternalInput")
    bkc = nc.dram_tensor("bkc", [P, 8], F32, kind="ExternalInput")
    bvr = nc.dram_tensor("bvr", [1, HK], BF, kind="ExternalInput")
    bor = nc.dram_tensor("bor", [1, D], BF, kind="ExternalInput")
    out = nc.dram_tensor("out", [SQ, D], F32, kind="ExternalOutput")

    qhT_d = nc.dram_tensor("qhT_scr", [HK, SQ], F8)
    khT_d = nc.dram_tensor("khT_scr", [HK, SK], F8)
    rden_d = nc.dram_tensor("rden_scr", [H, SQ], F32)

    with tile.TileContext(nc) as tc:
        with tc.tile_pool(name="persist", bufs=1) as pers:
            msk_sb = pers.tile([P, 16, SQ], U16, tag="msk")
            vaug = pers.tile([P, 16, 65 * H], BF, tag="vaug")
            ao_sb = pers.tile([P, 8, SQ], BF, tag="ao")
            wo_sb = pers.tile([P, 8, D], BF, tag="wo")
            ones16 = pers.tile([P, SQ], BF, tag="ones16")
            bor_sb = pers.tile([1, D], BF, tag="bor")

            # mask load split in quarters so attention can start early
            for quar in range(4):
                nc.sync.dma_start(
                    msk_sb[:, ds(4 * quar, 4), :],
                    mskT.ap().rearrange("(t p) s -> p t s", p=P)[
                        :, ds(4 * quar, 4), :
                    ],
                )
            nc.sync.dma_start(bor_sb[:], bor.ap())
            for _j in range(8):
                nc.sync.dma_start(
                    wo_sb[:, _j],
                    wo16.ap().rearrange("(j p) m -> p j m", p=P)[:, _j],
                )
            nc.vector.memset(ones16[:], 1.0)
            nc.gpsimd.memset(vaug[:].rearrange("p t c -> p (t c)"), 1.0)

            # ---------------- Q/K/V projections ----------------
            with (
                tc.tile_pool(name="wp", bufs=1) as wp,
                tc.tile_pool(name="xp", bufs=1) as xp,
                tc.tile_pool(name="pp", bufs=4, space="PSUM") as pp,
                tc.tile_pool(name="stg", bufs=3) as stg,
            ):
                wq_sb = wp.tile([P, 8, HK], F8, tag="wq")
                wk_sb = wp.tile([P, 8, HK], F8, tag="wk")
                wv_sb = wp.tile([P, 8, HK], BF, tag="wv")
                bvr_sb = wp.tile([1, HK], BF, tag="bvr")
                bq_sb = wp.tile([P, 8], F32, tag="bq")
                bk_sb = wp.tile([P, 8], F32, tag="bk")
                qx = xp.tile([P, 8, SQ], F8, tag="qx")
                kx = xp.tile([P, 8, SK], F8, tag="kx")
                for _j in range(8):
                    nc.sync.dma_start(
                        wq_sb[:, _j],
                        wq8.ap().rearrange("(j p) m -> p j m", p=P)[:, _j],
                    )
                    nc.sync.dma_start(
                        wk_sb[:, _j],
                        wk8.ap().rearrange("(j p) m -> p j m", p=P)[:, _j],
                    )
                    nc.sync.dma_start(
                        wv_sb[:, _j],
                        wv16.ap().rearrange("(j p) m -> p j m", p=P)[:, _j],
                    )
                    nc.sync.dma_start(
                        qx[:, _j],
                        qT8.ap().rearrange("(j p) s -> p j s", p=P)[:, _j],
                    )
                    nc.sync.dma_start(
                        kx[:, _j],
                        kT8.ap().rearrange("(j p) s -> p j s", p=P)[:, _j],
                    )
                nc.sync.dma_start(bvr_sb[:], bvr.ap())
                nc.sync.dma_start(bq_sb[:], bqc.ap())
                nc.sync.dma_start(bk_sb[:], bkc.ap())

                # Q projection -> qhT_d (fp8, value 8*qh)
                for i in range(8):
                    for ct in range(SQ // 512):
                        ps = pp.tile([P, 512], F32, tag="pp", name=f"psq{i}_{ct}")
                        for cc in range(2):
                            for u in range(4):
                                nc.tensor.matmul(
                                    ps[:, ds(256 * cc, 256)],
                                    wq_sb[:, ds(2 * u, 2), ts(i, P)],
                                    qx[:, ds(2 * u, 2), ds(512 * ct + 256 * cc, 256)],
                                    start=(u == 0),
                                    stop=(u == 3),
                                    perf_mode=PM.DoubleRow,
                                )
                        st = stg.tile([P, 512], F8, tag="stg")
                        nc.vector.tensor_scalar(
                            st[:], ps[:], 0.25, bq_sb[:, i : i + 1], MULT, ADD
                        )
                        nc.sync.dma_start(qhT_d.ap()[ts(i, P), ts(ct, 512)], st[:])

                # K projection -> khT_d (fp8, value 8*kh)
                for i in range(8):
                    for ct in range(SK // 512):
                        ps = pp.tile([P, 512], F32, tag="pp", name=f"psk{i}_{ct}")
                        for cc in range(2):
                            for u in range(4):
                                nc.tensor.matmul(
                                    ps[:, ds(256 * cc, 256)],
                                    wk_sb[:, ds(2 * u, 2), ts(i, P)],
                                    kx[:, ds(2 * u, 2), ds(512 * ct + 256 * cc, 256)],
                                    start=(u == 0),
                                    stop=(u == 3),
                                    perf_mode=PM.DoubleRow,
                                )
                        st = stg.tile([P, 512], F8, tag="stg")
                        nc.vector.tensor_scalar(
                            st[:], ps[:], 0.25, bk_sb[:, i : i + 1], MULT, ADD
                        )
                        nc.sync.dma_start(khT_d.ap()[ts(i, P), ts(ct, 512)], st[:])

                # V projection (swapped operands, bf16): psum [sk-tile,
                # hk-chunk] -> copy into vaug (ones cols pre-set by memset)
                with tc.tile_pool(name="vxp", bufs=2) as vxp:
                    for t in range(16):
                        vx = vxp.tile([P, 8, P], BF, tag="vx", name=f"vx{t}")
                        nc.sync.dma_start(
                            vx[:],
                            vT16.ap().rearrange("(j p) s -> p j s", p=P)[
                                :, :, ts(t, P)
                            ],
                        )
                        for c in range(2):
                            ps = pp.tile([P, 512], F32, tag="pp", name=f"psv{t}_{c}")
                            for u in range(8):
                                nc.tensor.matmul(
                                    ps[:],
                                    vx[:, u, :],
                                    wv_sb[:, u, ds(512 * c, 512)],
                                    start=(u == 0),
                                    stop=False,
                                )
                            nc.tensor.matmul(
                                ps[:],
                                ones16[0:1, 0:P],
                                bvr_sb[:, ds(512 * c, 512)],
                                start=False,
                                stop=True,
                            )
                            nc.vector.tensor_copy(
                                vaug[:, t, ds(520 * c, 520)]
                                .rearrange("p (h x) -> p h x", x=65)[:, :, 0:64],
                                ps[:].rearrange("p (h x) -> p h x", x=64),
                            )

            # ---------------- attention ----------------
            with (
                tc.tile_pool(name="qkp", bufs=2) as qkp,
                tc.tile_pool(name="ep", bufs=3) as ep,
                tc.tile_pool(name="sps", bufs=2, space="PSUM") as sps,
                tc.tile_pool(name="avp", bufs=1, space="PSUM") as avp,
                tc.tile_pool(name="totp", bufs=2) as totp,
                tc.tile_pool(name="rbp", bufs=2) as rbp,
            ):
                qk_tiles = {}
                def load_head(h):
                    qhs = qkp.tile([32, 2, SQ], F8, tag="qhs", name=f"qhs{h}")
                    khs = qkp.tile([32, 2, SK], F8, tag="khs", name=f"khs{h}")
                    nc.sync.dma_start(
                        qhs[:],
                        qhT_d.ap().rearrange(
                            "(h half p) s -> p h half s", half=2, p=32
                        )[:, h],
                    )
                    nc.sync.dma_start(
                        khs[:],
                        khT_d.ap().rearrange(
                            "(h half p) s -> p h half s", half=2, p=32
                        )[:, h],
                    )
                    qk_tiles[h] = (qhs, khs)

                load_head(0)
                load_head(1)
                for h in range(H):
                    qhs, khs = qk_tiles.pop(h)
                    if h + 2 < H:
                        load_head(h + 2)
                    pso = avp.tile([65, SQ], F32, tag="pso", name=f"pso{h}")
                    for t in range(16):
                        ps = sps.tile([P, SQ], F32, tag="sps", name=f"s{h}_{t}")
                        for c in range(4):
                            nc.tensor.matmul(
                                ps[:, ds(256 * c, 256)],
                                khs[:, :, ts(t, P)],
                                qhs[:, :, ds(256 * c, 256)],
                                start=True,
                                stop=True,
                                perf_mode=PM.DoubleRow,
                            )
                        e = ep.tile([P, SQ], BF, tag="e", name=f"e{h}_{t}")
                        nc.scalar.activation(e[:], ps[:], AF.Exp, scale=2.0 ** -9)
                        nc.vector.copy_predicated(e[:], msk_sb[:, t, :], ones16[:])
                        for c2 in range(2):
                            nc.tensor.matmul(
                                pso[:, ds(512 * c2, 512)],
                                vaug[:, t, ds(65 * h, 65)],
                                e[:, ds(512 * c2, 512)],
                                start=(t == 0),
                                stop=(t == 15),
                            )
                    tot = totp.tile([65, SQ], F32, tag="tot", name=f"tot{h}")
                    nc.vector.tensor_copy(tot[:], pso[:])
                    rcp = rbp.tile([1, SQ], F32, tag="rcp", name=f"rcp{h}")
                    nc.vector.reciprocal(rcp[:], tot[64:65, :])
                    nc.sync.dma_start(rden_d.ap()[h : h + 1, :], rcp[:])
                    rb = rbp.tile([64, SQ], F32, tag="rb", name=f"rb{h}")
                    nc.sync.dma_start(
                        rb[:], rden_d.ap()[h : h + 1, :].to_broadcast((64, SQ))
                    )
                    nc.vector.scalar_tensor_tensor(
                        ao_sb[ds(64 * (h % 2), 64), h // 2, :],
                        tot[0:64, :],
                        1.0,
                        rb[:],
                        MULT,
                        MULT,
                    )

            # ---------------- output projection ----------------
            with (
                tc.tile_pool(name="pp2", bufs=4, space="PSUM") as pp2,
                tc.tile_pool(name="ost", bufs=3) as ost,
            ):
                for s in range(8):
                    for c in range(2):
                        ps = pp2.tile([P, 512], F32, tag="op", name=f"po{s}_{c}")
                        for g in range(8):
                            nc.tensor.matmul(
                                ps[:],
                                ao_sb[:, g, ts(s, P)],
                                wo_sb[:, g, ds(512 * c, 512)],
                                start=(g == 0),
                                stop=False,
                            )
                        nc.tensor.matmul(
                            ps[:],
                            ones16[0:1, 0:P],
                            bor_sb[:, ds(512 * c, 512)],
                            start=False,
                            stop=True,
                        )
                        oo = ost.tile([P, 512], F32, tag="oo")
                        nc.scalar.activation(oo[:], ps[:], AF.Copy, scale=1.0)
                        nc.sync.dma_start(out.ap()[ts(s, P), ds(512 * c, 512)], oo[:])

    nc.compile()
    return nc


def make_host_inputs(q, k, v, mask, Wq, bq, Wk, bk, Wv, bv, Wo, bo):
    """Full inputs -> list of 8 per-core input dicts."""
    q = np.asarray(q, np.float32)
    k = np.asarray(k, np.float32)
    v = np.asarray(v, np.float32)
    mask = np.asarray(mask)

    def f8(a):
        return np.ascontiguousarray(a, dtype=np.float32).astype(NF8)

    def bf(a):
        return np.ascontiguousarray(a, dtype=np.float32).astype(NBF)

    shared = {
        "wq8": f8(32.0 * np.asarray(Wq, np.float32).transpose(1, 0, 2).reshape(D, HK)),
        "wk8": f8(32.0 * np.asarray(Wk, np.float32).transpose(1, 0, 2).reshape(D, HK)),
        "wv16": bf(np.asarray(Wv, np.float32).transpose(1, 0, 2).reshape(D, HK)),
        "wo16": bf(np.asarray(Wo, np.float32)),
        "bqc": np.ascontiguousarray(
            8.0 * np.asarray(bq, np.float32).reshape(HK).reshape(8, P).T
        ),
        "bkc": np.ascontiguousarray(
            8.0 * np.asarray(bk, np.float32).reshape(HK).reshape(8, P).T
        ),
        "bvr": bf(np.asarray(bv, np.float32).reshape(1, HK)),
        "bor": bf(np.asarray(bo, np.float32).reshape(1, D)),
    }

    in_maps = []
    for core in range(8):
        b, j = divmod(core, 2)
        qs = q[b, j * SQ : (j + 1) * SQ, :]
        ms = mask[b, j * SQ : (j + 1) * SQ, :]
        m = dict(shared)
        m["qT8"] = f8(qs.T)
        m["kT8"] = f8(k[b].T)
        m["vT16"] = bf(v[b].T)
        m["mskT"] = np.ascontiguousarray((~ms).T).astype(np.uint16)
        in_maps.append(m)
    return in_maps


def assemble_output(results):
    """8 per-core out [SQ, D] -> full [4, 2048, 1024]."""
    B, S = 4, 2048
    full = np.empty((B, S, D), np.float32)
    for core, res in enumerate(results):
        b, j = divmod(core, 2)
        full[b, j * SQ : (j + 1) * SQ, :] = res["out"]
    return full


class CompiledSpmd:
    def __init__(self, nc: bass.Bass, n_cores: int):
        bass2jax.install_neuronx_cc_hook()
        assert nc.dbg_addr is None, "build with debug=False"
        partition_name = (
            nc.partition_id_tensor.name if nc.partition_id_tensor else None
        )
        in_names, out_names, out_avals, zero_outs = [], [], [], []
        for alloc in nc.m.functions[0].allocations:
            if not isinstance(alloc, mybir.MemoryLocationSet):
                continue
            name = alloc.memorylocations[0].name
            if alloc.kind == "ExternalInput":
                if name != partition_name:
                    in_names.append(name)
            elif alloc.kind == "ExternalOutput":
                shape = tuple(alloc.tensor_shape)
                dtype = mybir.dt.np(alloc.dtype)
                out_names.append(name)
                out_avals.append(jax.core.ShapedArray(shape, dtype))
                zero_outs.append(np.zeros(shape, dtype))
        n_params = len(in_names)
        n_outs = len(out_avals)
        all_in_names = list(in_names) + list(out_names)
        if partition_name is not None:
            all_in_names.append(partition_name)

        def _body(*args):
            operands = list(args)
            if partition_name is not None:
                operands.append(bass2jax.partition_id_tensor())
            outs = bass2jax._bass_exec_p.bind(
                *operands,
                out_avals=tuple(out_avals),
                in_names=tuple(all_in_names),
                out_names=tuple(out_names),
                lowering_input_output_aliases=(),
                sim_require_finite=True,
                sim_require_nnan=True,
                nc=nc,
            )
            return tuple(outs)

        devices = jax.devices()[:n_cores]
        assert len(devices) == n_cores
        mesh = Mesh(np.asarray(devices), ("core",))
        self._mesh = mesh
        donate = tuple(range(n_params, n_params + n_outs))
        self._sharded = jax.jit(
            shard_map(
                _body,
                mesh=mesh,
                in_specs=(PartitionSpec("core"),) * (n_params + n_outs),
                out_specs=(PartitionSpec("core"),) * n_outs,
                check_rep=False,
            ),
            donate_argnums=donate,
            keep_unused=True,
        )
        self.in_names = in_names
        self.out_names = out_names
        self.out_avals = out_avals
        self.zero_outs = zero_outs
        self.n_cores = n_cores

    def _concat_inputs(self, in_maps):
        per_core = [[np.asarray(m[n]) for n in self.in_names] for m in in_maps]
        return [
            np.concatenate([per_core[c][i] for c in range(self.n_cores)], axis=0)
            for i in range(len(self.in_names))
        ]

    def run(self, in_maps, repeats: int = 1):
        """Returns (results_per_core, wall_times_s list of len repeats)."""
        from jax.sharding import NamedSharding

        mesh = self._mesh
        shard = NamedSharding(mesh, PartitionSpec("core"))
        concat_in = [
            jax.device_put(a, shard) for a in self._concat_inputs(in_maps)
        ]
        rep_zeros = [
            [
                jax.device_put(
                    np.zeros((self.n_cores * z.shape[0], *z.shape[1:]), z.dtype),
                    shard,
                )
                for z in self.zero_outs
            ]
            for _ in range(repeats)
        ]
        jax.block_until_ready(concat_in)
        jax.block_until_ready(rep_zeros)
        times = []
        out_arrs = None
        for r in range(repeats):
            t0 = time.perf_counter()
            out_arrs = self._sharded(*concat_in, *rep_zeros[r])
            jax.block_until_ready(out_arrs)
            times.append(time.perf_counter() - t0)
        results = [
            {
                name: np.asarray(out_arrs[i]).reshape(
                    self.n_cores, *self.out_avals[i].shape
                )[c]
                for i, name in enumerate(self.out_names)
            }
            for c in range(self.n_cores)
        ]
        return results, times


_COMPILED = None


def _get_compiled():
    global _COMPILED
    if _COMPILED is None:
        nc = build_mha()
        _COMPILED = CompiledSpmd(nc, 8)
    return _COMPILED


def kernel(**inputs) -> np.ndarray:
    comp = _get_compiled()
    in_maps = make_host_inputs(**inputs)
    results, _ = comp.run(in_maps, repeats=1)
    return assemble_output(results)


# revision 13
# speedup vs baseline: 1.2138x; 1.0314x over previous
"""Self-contained Trainium2 Bass kernel for nn_MultiHeadAttention_68367289417808.

kernel(**inputs) takes FULL unsharded inputs (as in reference.setup_inputs())
and returns the FULL [4, 2048, 1024] output.

Sharding: 8 cores = (batch 4) x (query-half 2); no collectives needed.

Per-core pipeline (mixed precision, tuned against the TRN2 cost model):
  - Q/K projections in fp8e4m3 DoubleRow (weights x32, outputs stored as
    8*qh in fp8), V projection in fp8e4m3 DoubleRow with vh stored bf16.
  - scores = kh^T qh per head via fp8 DoubleRow (dk=64 split into two
    32-row k-tiles living on the same 32 partitions).
  - exp on the Act engine straight out of PSUM into bf16 (scale 2^-9
    compensates the 8x8 operand scaling); masked positions are then
    overwritten with exactly 1.0 (= exp(-1e-6) to ulp) via one DVE
    copy_predicated against a ones tile, matching the reference's
    masked_fill(-1e-6) semantics with no correction terms.
  - AV + softmax denominator (ones column in vaug) in bf16 matmuls,
    normalize on DVE (reciprocal + scalar_tensor_tensor), output
    projection + bias in bf16 with the final copy on the Act engine.
"""
import time

import jax
import ml_dtypes
import numpy as np
from jax.experimental.shard_map import shard_map
from jax.sharding import Mesh, PartitionSpec

import concourse.bass as bass
import concourse.bacc as bacc
import concourse.mybir as mybir
import concourse.tile as tile
from concourse import bass2jax
from concourse.bass import ts, ds

F32 = mybir.dt.float32
F8 = mybir.dt.float8e4
BF = mybir.dt.bfloat16
U16 = mybir.dt.uint16
FR = mybir.dt.float32r
AF = mybir.ActivationFunctionType
PM = mybir.MatmulPerfMode
MULT = mybir.AluOpType.mult
ADD = mybir.AluOpType.add

NF8 = ml_dtypes.float8_e4m3
NBF = ml_dtypes.bfloat16

P = 128
SQ = 1024
SK = 2048
D = 1024
H = 16
DK = 64
HK = 1024


def build_mha(phases=('proj', 'attn', 'oproj')):
    nc = bacc.Bacc("TRN2", target_bir_lowering=False)

    qT8 = nc.dram_tensor("qT8", [D, SQ], F8, kind="ExternalInput")
    kT8 = nc.dram_tensor("kT8", [D, SK], F8, kind="ExternalInput")
    vT16 = nc.dram_tensor("vT16", [D, SK], BF, kind="ExternalInput")
    mskT = nc.dram_tensor("mskT", [SK, SQ], U16, kind="ExternalInput")
    wq8 = nc.dram_tensor("wq8", [D, HK], F8, kind="ExternalInput")
    wk8 = nc.dram_tensor("wk8", [D, HK], F8, kind="ExternalInput")
    wv16 = nc.dram_tensor("wv16", [D, HK], BF, kind="ExternalInput")
    wo16 = nc.dram_tensor("wo16", [HK, D], BF, kind="ExternalInput")
    bqc = nc.dram_tensor("bqc", [P, 8], F32, kind="ExternalInput")
    bkc = nc.dram_tensor("bkc", [P, 8], F32, kind="ExternalInput")
    bvr = nc.dram_tensor("bvr", [1, HK], BF, kind="ExternalInput")
    bor = nc.dram_tensor("bor", [1, D], BF, kind="ExternalInput")
    out = nc.dram_tensor("out", [SQ, D], F32, kind="ExternalOutput")

    qhT_d = nc.dram_tensor("qhT_scr", [HK, SQ], F8)
    khT_d = nc.dram_tensor("khT_scr", [HK, SK], F8)
    rden_d = nc.dram_tensor("rden_scr", [H, SQ], F32)

    with tile.TileContext(nc) as tc:
        with tc.tile_pool(name="persist", bufs=1) as pers:
            msk_sb = pers.tile([P, 16, SQ], U16, tag="msk")
            vaug = pers.tile([P, 16, 65 * H], BF, tag="vaug")
            ao_sb = pers.tile([P, 8, SQ], BF, tag="ao")
            wo_sb = pers.tile([P, 8, D], BF, tag="wo")
            ones16 = pers.tile([P, SQ], BF, tag="ones16")
            bor_sb = pers.tile([1, D], BF, tag="bor")

            nc.vector.memset(ones16[:], 1.0)
            nc.gpsimd.memset(vaug[:].rearrange("p t (h x) -> p t h x", x=65)[:, :, :, 64:65], 1.0)

            # ---------------- Q/K/V projections ----------------
            with (
                tc.tile_pool(name="wp", bufs=1) as wp,
                tc.tile_pool(name="xp", bufs=1) as xp,
                tc.tile_pool(name="pp", bufs=4, space="PSUM") as pp,
                tc.tile_pool(name="stg", bufs=3) as stg,
            ):
                wq_sb = wp.tile([P, 8, HK], F8, tag="wq")
                wk_sb = wp.tile([P, 8, HK], F8, tag="wk")
                wv_sb = wp.tile([P, 8, HK], BF, tag="wv")
                bvr_sb = wp.tile([1, HK], BF, tag="bvr")
                bq_sb = wp.tile([P, 8], F32, tag="bq")
                bk_sb = wp.tile([P, 8], F32, tag="bk")
                qx = xp.tile([P, 8, SQ], F8, tag="qx")
                kx = xp.tile([P, 8, SK], F8, tag="kx")
                vx = xp.tile([P, 8, SK], BF, tag="vx")
                for _j in range(8):
                    nc.sync.dma_start(
                        wq_sb[:, _j],
                        wq8.ap().rearrange("(j p) m -> p j m", p=P)[:, _j],
                    )
                    nc.sync.dma_start(
                        wk_sb[:, _j],
                        wk8.ap().rearrange("(j p) m -> p j m", p=P)[:, _j],
                    )
                    nc.sync.dma_start(
                        wv_sb[:, _j],
                        wv16.ap().rearrange("(j p) m -> p j m", p=P)[:, _j],
                    )
                    nc.sync.dma_start(
                        qx[:, _j],
                        qT8.ap().rearrange("(j p) s -> p j s", p=P)[:, _j],
                    )
                    nc.sync.dma_start(
                        kx[:, _j],
                        kT8.ap().rearrange("(j p) s -> p j s", p=P)[:, _j],
                    )
                    nc.sync.dma_start(
                        vx[:, _j],
                        vT16.ap().rearrange("(j p) s -> p j s", p=P)[:, _j],
                    )
                nc.sync.dma_start(bvr_sb[:], bvr.ap())
                nc.sync.dma_start(bq_sb[:], bqc.ap())
                nc.sync.dma_start(bk_sb[:], bkc.ap())

                # Q projection -> qhT_d (fp8, value 8*qh)
                for i in range(8 if ('proj' in phases or 'q' in phases) else 0):
                    for ct in range(SQ // 512):
                        ps = pp.tile([P, 512], F32, tag="pp", name=f"psq{i}_{ct}")
                        for cc in range(2):
                            for u in range(4):
                                nc.tensor.matmul(
                                    ps[:, ds(256 * cc, 256)],
                                    wq_sb[:, ds(2 * u, 2), ts(i, P)],
                                    qx[:, ds(2 * u, 2), ds(512 * ct + 256 * cc, 256)],
                                    start=(u == 0),
                                    stop=(u == 3),
                                    perf_mode=PM.DoubleRow,
                                )
                        st = stg.tile([P, 512], F8, tag="stg")
                        nc.vector.tensor_scalar(
                            st[:], ps[:], 0.25, bq_sb[:, i : i + 1], MULT, ADD
                        )
                        nc.sync.dma_start(qhT_d.ap()[ts(i, P), ts(ct, 512)], st[:])

                # K projection -> khT_d (fp8, value 8*kh)
                for i in range(8 if ('proj' in phases or 'k' in phases) else 0):
                    for ct in range(SK // 512):
                        ps = pp.tile([P, 512], F32, tag="pp", name=f"psk{i}_{ct}")
                        for cc in range(2):
                            for u in range(4):
                                nc.tensor.matmul(
                                    ps[:, ds(256 * cc, 256)],
                                    wk_sb[:, ds(2 * u, 2), ts(i, P)],
                                    kx[:, ds(2 * u, 2), ds(512 * ct + 256 * cc, 256)],
                                    start=(u == 0),
                                    stop=(u == 3),
                                    perf_mode=PM.DoubleRow,
                                )
                        st = stg.tile([P, 512], F8, tag="stg")
                        nc.vector.tensor_scalar(
                            st[:], ps[:], 0.25, bk_sb[:, i : i + 1], MULT, ADD
                        )
                        nc.sync.dma_start(khT_d.ap()[ts(i, P), ts(ct, 512)], st[:])

                # V projection (swapped operands, bf16): psum [sk-tile,
                # hk-chunk] -> copy into vaug (ones cols pre-set by memset)
                for t in range(16 if ('proj' in phases or 'v' in phases) else 0):
                    for c in range(2):
                        ps = pp.tile([P, 512], F32, tag="pp", name=f"psv{t}_{c}")
                        for u in range(8):
                            nc.tensor.matmul(
                                ps[:],
                                vx[:, u, ts(t, P)],
                                wv_sb[:, u, ds(512 * c, 512)],
                                start=(u == 0),
                                stop=False,
                            )
                        nc.tensor.matmul(
                            ps[:],
                            ones16[0:1, 0:P],
                            bvr_sb[:, ds(512 * c, 512)],
                            start=False,
                            stop=True,
                        )
                        nc.vector.tensor_copy(
                            vaug[:, t, ds(520 * c, 520)]
                            .rearrange("p (h x) -> p h x", x=65)[:, :, 0:64],
                            ps[:].rearrange("p (h x) -> p h x", x=64),
                        )

            # deferred big loads: mask (quartered) + Wo + bo ride the DMA
            # queues while the projections run
            for quar in range(4):
                nc.sync.dma_start(
                    msk_sb[:, ds(4 * quar, 4), :],
                    mskT.ap().rearrange("(t p) s -> p t s", p=P)[
                        :, ds(4 * quar, 4), :
                    ],
                )
            nc.sync.dma_start(bor_sb[:], bor.ap())
            for _j in range(8):
                nc.sync.dma_start(
                    wo_sb[:, _j],
                    wo16.ap().rearrange("(j p) m -> p j m", p=P)[:, _j],
                )

            # ---------------- attention ----------------
            if 'attn' not in phases:
                nc.compile()
                return nc
            with (
                tc.tile_pool(name="qkp", bufs=2) as qkp,
                tc.tile_pool(name="ep", bufs=3) as ep,
                tc.tile_pool(name="sps", bufs=2, space="PSUM") as sps,
                tc.tile_pool(name="avp", bufs=1, space="PSUM") as avp,
                tc.tile_pool(name="totp", bufs=2) as totp,
                tc.tile_pool(name="rbp", bufs=2) as rbp,
                tc.tile_pool(name="rbps", bufs=1, space="PSUM") as rbps,
            ):
                onesr = rbp.tile([1, 64], FR, tag="onesr")
                nc.vector.tensor_copy(onesr[:], ones16[0:1, 0:64])
                qk_tiles = {}
                def load_head(h):
                    qhs = qkp.tile([32, 2, SQ], F8, tag="qhs", name=f"qhs{h}")
                    khs = qkp.tile([32, 2, SK], F8, tag="khs", name=f"khs{h}")
                    nc.sync.dma_start(
                        qhs[:],
                        qhT_d.ap().rearrange(
                            "(h half p) s -> p h half s", half=2, p=32
                        )[:, h],
                    )
                    nc.sync.dma_start(
                        khs[:],
                        khT_d.ap().rearrange(
                            "(h half p) s -> p h half s", half=2, p=32
                        )[:, h],
                    )
                    qk_tiles[h] = (qhs, khs)

                load_head(0)
                load_head(1)
                pending_norm = [None]
                for h in range(H):
                    qhs, khs = qk_tiles.pop(h)
                    if h + 2 < H:
                        load_head(h + 2)
                    pso = avp.tile([65, SQ], F32, tag="pso", name=f"pso{h}")
                    for t in range(16):
                        if t == 2 and pending_norm[0] is not None:
                            pending_norm[0]()
                            pending_norm[0] = None
                        ps = sps.tile([P, SQ], F32, tag="sps", name=f"s{h}_{t}")
                        for c in range(4):
                            nc.tensor.matmul(
                                ps[:, ds(256 * c, 256)],
                                khs[:, :, ts(t, P)],
                                qhs[:, :, ds(256 * c, 256)],
                                start=True,
                                stop=True,
                                perf_mode=PM.DoubleRow,
                            )
                        e = ep.tile([P, SQ], BF, tag="e", name=f"e{h}_{t}")
                        nc.scalar.activation(e[:], ps[:], AF.Exp, scale=2.0 ** -9)
                        nc.vector.copy_predicated(e[:], msk_sb[:, t, :], ones16[:])
                        for c2 in range(2):
                            nc.tensor.matmul(
                                pso[:, ds(512 * c2, 512)],
                                vaug[:, t, ds(65 * h, 65)],
                                e[:, ds(512 * c2, 512)],
                                start=(t == 0),
                                stop=(t == 15),
                            )
                    def make_norm(h, pso):
                        def emit():
                            tot = totp.tile([65, SQ], F32, tag="tot", name=f"tot{h}")
                            nc.vector.tensor_copy(tot[:], pso[:])
                            rcp = rbp.tile([1, SQ], FR, tag="rcp", name=f"rcp{h}")
                            with nc.allow_low_precision(reason="f32r recip, multiplicative use"):
                                nc.vector.reciprocal(rcp[:], tot[64:65, :])
                            rb = rbps.tile([64, SQ], F32, tag="rb", name=f"rb{h}")
                            for _cb in range(2):
                                nc.tensor.matmul(
                                    rb[:, ds(512 * _cb, 512)],
                                    onesr[:, 0:64],
                                    rcp[:, ds(512 * _cb, 512)],
                                    start=True,
                                    stop=True,
                                )
                            nc.vector.scalar_tensor_tensor(
                                ao_sb[ds(64 * (h % 2), 64), h // 2, :],
                                tot[0:64, :],
                                1.0,
                                rb[:],
                                MULT,
                                MULT,
                            )
                        return emit
                    pending_norm[0] = make_norm(h, pso)
                pending_norm[0]()

            # ---------------- output projection ----------------
            if 'oproj' not in phases:
                nc.compile()
                return nc
            with (
                tc.tile_pool(name="pp2", bufs=4, space="PSUM") as pp2,
                tc.tile_pool(name="ost", bufs=3) as ost,
            ):
                for s in range(8):
                    for c in range(2):
                        ps = pp2.tile([P, 512], F32, tag="op", name=f"po{s}_{c}")
                        for g in range(8):
                            nc.tensor.matmul(
                                ps[:],
                                ao_sb[:, g, ts(s, P)],
                                wo_sb[:, g, ds(512 * c, 512)],
                                start=(g == 0),
                                stop=False,
                            )
                        nc.tensor.matmul(
                            ps[:],
                            ones16[0:1, 0:P],
                            bor_sb[:, ds(512 * c, 512)],
                            start=False,
                            stop=True,
                        )
                        oo = ost.tile([P, 512], F32, tag="oo")
                        nc.scalar.activation(oo[:], ps[:], AF.Copy, scale=1.0)
                        nc.sync.dma_start(out.ap()[ts(s, P), ds(512 * c, 512)], oo[:])

    nc.compile()
    return nc


def make_host_inputs(q, k, v, mask, Wq, bq, Wk, bk, Wv, bv, Wo, bo):
    """Full inputs -> list of 8 per-core input dicts."""
    q = np.asarray(q, np.float32)
    k = np.asarray(k, np.float32)
    v = np.asarray(v, np.float32)
    mask = np.asarray(mask)

    def f8(a):
        return np.ascontiguousarray(a, dtype=np.float32).astype(NF8)

    def bf(a):
        return np.ascontiguousarray(a, dtype=np.float32).astype(NBF)

    shared = {
        "wq8": f8(32.0 * np.asarray(Wq, np.float32).transpose(1, 0, 2).reshape(D, HK)),
        "wk8": f8(32.0 * np.asarray(Wk, np.float32).transpose(1, 0, 2).reshape(D, HK)),
        "wv16": bf(np.asarray(Wv, np.float32).transpose(1, 0, 2).reshape(D, HK)),
        "wo16": bf(np.asarray(Wo, np.float32)),
        "bqc": np.ascontiguousarray(
            8.0 * np.asarray(bq, np.float32).reshape(HK).reshape(8, P).T
        ),
        "bkc": np.ascontiguousarray(
            8.0 * np.asarray(bk, np.float32).reshape(HK).reshape(8, P).T
        ),
        "bvr": bf(np.asarray(bv, np.float32).reshape(1, HK)),
        "bor": bf(np.asarray(bo, np.float32).reshape(1, D)),
    }

    in_maps = []
    for core in range(8):
        b, j = divmod(core, 2)
        qs = q[b, j * SQ : (j + 1) * SQ, :]
        ms = mask[b, j * SQ : (j + 1) * SQ, :]
        m = dict(shared)
        m["qT8"] = f8(qs.T)
        m["kT8"] = f8(k[b].T)
        m["vT16"] = bf(v[b].T)
        m["mskT"] = np.ascontiguousarray((~ms).T).astype(np.uint16)
        in_maps.append(m)
    return in_maps


def assemble_output(results):
    """8 per-core out [SQ, D] -> full [4, 2048, 1024]."""
    B, S = 4, 2048
    full = np.empty((B, S, D), np.float32)
    for core, res in enumerate(results):
        b, j = divmod(core, 2)
        full[b, j * SQ : (j + 1) * SQ, :] = res["out"]
    return full


class CompiledSpmd:
    def __init__(self, nc: bass.Bass, n_cores: int):
        bass2jax.install_neuronx_cc_hook()
        assert nc.dbg_addr is None, "build with debug=False"
        partition_name = (
            nc.partition_id_tensor.name if nc.partition_id_tensor else None
        )
        in_names, out_names, out_avals, zero_outs = [], [], [], []
        for alloc in nc.m.functions[0].allocations:
            if not isinstance(alloc, mybir.MemoryLocationSet):
                continue
            name = alloc.memorylocations[0].name
            if alloc.kind == "ExternalInput":
                if name != partition_name:
                    in_names.append(name)
            elif alloc.kind == "ExternalOutput":
                shape = tuple(alloc.tensor_shape)
                dtype = mybir.dt.np(alloc.dtype)
                out_names.append(name)
                out_avals.append(jax.core.ShapedArray(shape, dtype))
                zero_outs.append(np.zeros(shape, dtype))
        n_params = len(in_names)
        n_outs = len(out_avals)
        all_in_names = list(in_names) + list(out_names)
        if partition_name is not None:
            all_in_names.append(partition_name)

        def _body(*args):
            operands = list(args)
            if partition_name is not None:
                operands.append(bass2jax.partition_id_tensor())
            outs = bass2jax._bass_exec_p.bind(
                *operands,
                out_avals=tuple(out_avals),
                in_names=tuple(all_in_names),
                out_names=tuple(out_names),
                lowering_input_output_aliases=(),
                sim_require_finite=True,
                sim_require_nnan=True,
                nc=nc,
            )
            return tuple(outs)

        devices = jax.devices()[:n_cores]
        assert len(devices) == n_cores
        mesh = Mesh(np.asarray(devices), ("core",))
        self._mesh = mesh
        donate = tuple(range(n_params, n_params + n_outs))
        self._sharded = jax.jit(
            shard_map(
                _body,
                mesh=mesh,
                in_specs=(PartitionSpec("core"),) * (n_params + n_outs),
                out_specs=(PartitionSpec("core"),) * n_outs,
                check_rep=False,
            ),
            donate_argnums=donate,
            keep_unused=True,
        )
        self.in_names = in_names
        self.out_names = out_names
        self.out_avals = out_avals
        self.zero_outs = zero_outs
        self.n_cores = n_cores

    def _concat_inputs(self, in_maps):
        per_core = [[np.asarray(m[n]) for n in self.in_names] for m in in_maps]
        return [
            np.concatenate([per_core[c][i] for c in range(self.n_cores)], axis=0)
            for i in range(len(self.in_names))
        ]

    def run(self, in_maps, repeats: int = 1):
        """Returns (results_per_core, wall_times_s list of len repeats)."""
        from jax.sharding import NamedSharding

        mesh = self._mesh
        shard = NamedSharding(mesh, PartitionSpec("core"))
        concat_in = [
            jax.device_put(a, shard) for a in self._concat_inputs(in_maps)
        ]
        rep_zeros = [
            [
                jax.device_put(
                    np.zeros((self.n_cores * z.shape[0], *z.shape[1:]), z.dtype),
                    shard,
                )
                for z in self.zero_outs
            ]
            for _ in range(repeats)
        ]
        jax.block_until_ready(concat_in)
        jax.block_until_ready(rep_zeros)
        times = []
        out_arrs = None
        for r in range(repeats):
            t0 = time.perf_counter()
            out_arrs = self._sharded(*concat_in, *rep_zeros[r])
            jax.block_until_ready(out_arrs)
            times.append(time.perf_counter() - t0)
        results = [
            {
                name: np.asarray(out_arrs[i]).reshape(
                    self.n_cores, *self.out_avals[i].shape
                )[c]
                for i, name in enumerate(self.out_names)
            }
            for c in range(self.n_cores)
        ]
        return results, times


_COMPILED = None


def _get_compiled():
    global _COMPILED
    if _COMPILED is None:
        nc = build_mha()
        _COMPILED = CompiledSpmd(nc, 8)
    return _COMPILED


def kernel(**inputs) -> np.ndarray:
    comp = _get_compiled()
    in_maps = make_host_inputs(**inputs)
    results, _ = comp.run(in_maps, repeats=1)
    return assemble_output(results)


# revision 21
# speedup vs baseline: 1.2146x; 1.0007x over previous
"""Self-contained Trainium2 Bass kernel for nn_MultiHeadAttention_68367289417808.

kernel(**inputs) takes FULL unsharded inputs (as in reference.setup_inputs())
and returns the FULL [4, 2048, 1024] output.

Sharding: 8 cores = (batch 4) x (query-half 2); no collectives needed.

Per-core pipeline (mixed precision, tuned against the TRN2 cost model):
  - Q/K projections in fp8e4m3 DoubleRow (weights x32, outputs stored as
    8*qh in fp8), V projection in fp8e4m3 DoubleRow with vh stored bf16.
  - scores = kh^T qh per head via fp8 DoubleRow (dk=64 split into two
    32-row k-tiles living on the same 32 partitions).
  - exp on the Act engine straight out of PSUM into bf16 (scale 2^-9
    compensates the 8x8 operand scaling); masked positions are then
    overwritten with exactly 1.0 (= exp(-1e-6) to ulp) via one DVE
    copy_predicated against a ones tile, matching the reference's
    masked_fill(-1e-6) semantics with no correction terms.
  - AV + softmax denominator (ones column in vaug) in bf16 matmuls,
    normalize on DVE (reciprocal + scalar_tensor_tensor), output
    projection + bias in bf16 with the final copy on the Act engine.
"""
import time

import jax
import ml_dtypes
import numpy as np
from jax.experimental.shard_map import shard_map
from jax.sharding import Mesh, PartitionSpec

import concourse.bass as bass
import concourse.bacc as bacc
import concourse.mybir as mybir
import concourse.tile as tile
from concourse import bass2jax
from concourse.bass import ts, ds

F32 = mybir.dt.float32
F8 = mybir.dt.float8e4
BF = mybir.dt.bfloat16
U16 = mybir.dt.uint16
FR = mybir.dt.float32r
AF = mybir.ActivationFunctionType
PM = mybir.MatmulPerfMode
MULT = mybir.AluOpType.mult
ADD = mybir.AluOpType.add

NF8 = ml_dtypes.float8_e4m3
NBF = ml_dtypes.bfloat16

P = 128
SQ = 1024
SK = 2048
D = 1024
H = 16
DK = 64
HK = 1024


def build_mha(phases=('proj', 'attn', 'oproj')):
    nc = bacc.Bacc("TRN2", target_bir_lowering=False)

    qT8 = nc.dram_tensor("qT8", [D, SQ], F8, kind="ExternalInput")
    kT8 = nc.dram_tensor("kT8", [D, SK], F8, kind="ExternalInput")
    vT16 = nc.dram_tensor("vT16", [D, SK], BF, kind="ExternalInput")
    mskT = nc.dram_tensor("mskT", [SK, SQ], U16, kind="ExternalInput")
    wq8 = nc.dram_tensor("wq8", [D, HK], F8, kind="ExternalInput")
    wk8 = nc.dram_tensor("wk8", [D, HK], F8, kind="ExternalInput")
    wv16 = nc.dram_tensor("wv16", [D, HK], BF, kind="ExternalInput")
    wo16 = nc.dram_tensor("wo16", [HK, D], BF, kind="ExternalInput")
    bqc = nc.dram_tensor("bqc", [P, 8], F32, kind="ExternalInput")
    bkc = nc.dram_tensor("bkc", [P, 8], F32, kind="ExternalInput")
    bvr = nc.dram_tensor("bvr", [1, HK], BF, kind="ExternalInput")
    bor = nc.dram_tensor("bor", [1, D], BF, kind="ExternalInput")
    out = nc.dram_tensor("out", [SQ, D], F32, kind="ExternalOutput")

    qhT_d = nc.dram_tensor("qhT_scr", [HK, SQ], F8)
    khT_d = nc.dram_tensor("khT_scr", [HK, SK], F8)
    rden_d = nc.dram_tensor("rden_scr", [H, SQ], F32)

    with tile.TileContext(nc) as tc:
        with tc.tile_pool(name="persist", bufs=1) as pers:
            msk_sb = pers.tile([P, 16, SQ], U16, tag="msk")
            vaug = pers.tile([P, 16, 65 * H], BF, tag="vaug")
            ao_sb = pers.tile([P, 8, SQ], BF, tag="ao")
            wo_sb = pers.tile([P, 8, D], BF, tag="wo")
            ones16 = pers.tile([P, SQ], BF, tag="ones16")
            bor_sb = pers.tile([1, D], BF, tag="bor")

            nc.vector.memset(ones16[:], 1.0)
            nc.gpsimd.memset(vaug[:].rearrange("p t c -> p (t c)"), 1.0)

            # ---------------- Q/K/V projections ----------------
            with (
                tc.tile_pool(name="wp", bufs=1) as wp,
                tc.tile_pool(name="xp", bufs=1) as xp,
                tc.tile_pool(name="pp", bufs=4, space="PSUM") as pp,
                tc.tile_pool(name="stg", bufs=3) as stg,
            ):
                wq_sb = wp.tile([P, 8, HK], F8, tag="wq")
                wk_sb = wp.tile([P, 8, HK], F8, tag="wk")
                wv_sb = wp.tile([P, 8, HK], BF, tag="wv")
                bvr_sb = wp.tile([1, HK], BF, tag="bvr")
                bq_sb = wp.tile([P, 8], F32, tag="bq")
                bk_sb = wp.tile([P, 8], F32, tag="bk")
                qx = xp.tile([P, 8, SQ], F8, tag="qx")
                kx = xp.tile([P, 8, SK], F8, tag="kx")
                vx = xp.tile([P, 8, SK], BF, tag="vx")
                for _j in range(8):
                    nc.sync.dma_start(
                        wq_sb[:, _j],
                        wq8.ap().rearrange("(j p) m -> p j m", p=P)[:, _j],
                    )
                    nc.sync.dma_start(
                        wk_sb[:, _j],
                        wk8.ap().rearrange("(j p) m -> p j m", p=P)[:, _j],
                    )
                    nc.sync.dma_start(
                        wv_sb[:, _j],
                        wv16.ap().rearrange("(j p) m -> p j m", p=P)[:, _j],
                    )
                    nc.sync.dma_start(
                        qx[:, _j],
                        qT8.ap().rearrange("(j p) s -> p j s", p=P)[:, _j],
                    )
                    nc.sync.dma_start(
                        kx[:, _j],
                        kT8.ap().rearrange("(j p) s -> p j s", p=P)[:, _j],
                    )
                    nc.sync.dma_start(
                        vx[:, _j],
                        vT16.ap().rearrange("(j p) s -> p j s", p=P)[:, _j],
                    )
                nc.sync.dma_start(bvr_sb[:], bvr.ap())
                nc.sync.dma_start(bq_sb[:], bqc.ap())
                nc.sync.dma_start(bk_sb[:], bkc.ap())

                # Q projection -> qhT_d (fp8, value 8*qh)
                for i in range(8 if ('proj' in phases or 'q' in phases) else 0):
                    for ct in range(SQ // 512):
                        ps = pp.tile([P, 512], F32, tag="pp", name=f"psq{i}_{ct}")
                        for cc in range(2):
                            for u in range(4):
                                nc.tensor.matmul(
                                    ps[:, ds(256 * cc, 256)],
                                    wq_sb[:, ds(2 * u, 2), ts(i, P)],
                                    qx[:, ds(2 * u, 2), ds(512 * ct + 256 * cc, 256)],
                                    start=(u == 0),
                                    stop=(u == 3),
                                    perf_mode=PM.DoubleRow,
                                )
                        st = stg.tile([P, 512], F8, tag="stg")
                        nc.vector.tensor_scalar(
                            st[:], ps[:], 0.25, bq_sb[:, i : i + 1], MULT, ADD
                        )
                        nc.sync.dma_start(qhT_d.ap()[ts(i, P), ts(ct, 512)], st[:])

                # K projection -> khT_d (fp8, value 8*kh)
                for i in range(8 if ('proj' in phases or 'k' in phases) else 0):
                    for ct in range(SK // 512):
                        ps = pp.tile([P, 512], F32, tag="pp", name=f"psk{i}_{ct}")
                        for cc in range(2):
                            for u in range(4):
                                nc.tensor.matmul(
                                    ps[:, ds(256 * cc, 256)],
                                    wk_sb[:, ds(2 * u, 2), ts(i, P)],
                                    kx[:, ds(2 * u, 2), ds(512 * ct + 256 * cc, 256)],
                                    start=(u == 0),
                                    stop=(u == 3),
                                    perf_mode=PM.DoubleRow,
                                )
                        st = stg.tile([P, 512], F8, tag="stg")
                        nc.vector.tensor_scalar(
                            st[:], ps[:], 0.25, bk_sb[:, i : i + 1], MULT, ADD
                        )
                        nc.sync.dma_start(khT_d.ap()[ts(i, P), ts(ct, 512)], st[:])

                # V projection (swapped operands, bf16): psum [sk-tile,
                # hk-chunk] -> copy into vaug (ones cols pre-set by memset)
                for t in range(16 if ('proj' in phases or 'v' in phases) else 0):
                    for c in range(2):
                        ps = pp.tile([P, 512], F32, tag="pp", name=f"psv{t}_{c}")
                        for u in range(8):
                            nc.tensor.matmul(
                                ps[:],
                                vx[:, u, ts(t, P)],
                                wv_sb[:, u, ds(512 * c, 512)],
                                start=(u == 0),
                                stop=False,
                            )
                        nc.tensor.matmul(
                            ps[:],
                            ones16[0:1, 0:P],
                            bvr_sb[:, ds(512 * c, 512)],
                            start=False,
                            stop=True,
                        )
                        nc.vector.tensor_copy(
                            vaug[:, t, ds(520 * c, 520)]
                            .rearrange("p (h x) -> p h x", x=65)[:, :, 0:64],
                            ps[:].rearrange("p (h x) -> p h x", x=64),
                        )

            # deferred big loads: mask (quartered) + Wo + bo ride the DMA
            # queues while the projections run
            for quar in range(4):
                nc.sync.dma_start(
                    msk_sb[:, ds(4 * quar, 4), :],
                    mskT.ap().rearrange("(t p) s -> p t s", p=P)[
                        :, ds(4 * quar, 4), :
                    ],
                )
            nc.sync.dma_start(bor_sb[:], bor.ap())
            for _j in range(8):
                nc.sync.dma_start(
                    wo_sb[:, _j],
                    wo16.ap().rearrange("(j p) m -> p j m", p=P)[:, _j],
                )

            # ---------------- attention ----------------
            if 'attn' not in phases:
                nc.compile()
                return nc
            with (
                tc.tile_pool(name="qkp", bufs=2) as qkp,
                tc.tile_pool(name="ep", bufs=4) as ep,
                tc.tile_pool(name="sps", bufs=2, space="PSUM") as sps,
                tc.tile_pool(name="avp", bufs=1, space="PSUM") as avp,
                tc.tile_pool(name="totp", bufs=2) as totp,
                tc.tile_pool(name="rbp", bufs=2) as rbp,
                tc.tile_pool(name="rbps", bufs=1, space="PSUM") as rbps,
            ):
                onesr = rbp.tile([1, 64], FR, tag="onesr")
                nc.vector.tensor_copy(onesr[:], ones16[0:1, 0:64])
                qk_tiles = {}
                def load_head(h):
                    qhs = qkp.tile([32, 2, SQ], F8, tag="qhs", name=f"qhs{h}")
                    khs = qkp.tile([32, 2, SK], F8, tag="khs", name=f"khs{h}")
                    nc.sync.dma_start(
                        qhs[:],
                        qhT_d.ap().rearrange(
                            "(h half p) s -> p h half s", half=2, p=32
                        )[:, h],
                    )
                    nc.sync.dma_start(
                        khs[:],
                        khT_d.ap().rearrange(
                            "(h half p) s -> p h half s", half=2, p=32
                        )[:, h],
                    )
                    qk_tiles[h] = (qhs, khs)

                load_head(0)
                load_head(1)
                pending = [None]
                for h in range(H):
                    qhs, khs = qk_tiles.pop(h)
                    if h + 2 < H:
                        load_head(h + 2)
                    pso = avp.tile([65, SQ], F32, tag="pso", name=f"pso{h}")
                    for t in range(16):
                        if t == 2 and pending[0] is not None:
                            pending[0]()
                            pending[0] = None
                        ps = sps.tile([P, SQ], F32, tag="sps", name=f"s{h}_{t}")
                        for c in range(4):
                            nc.tensor.matmul(
                                ps[:, ds(256 * c, 256)],
                                khs[:, :, ts(t, P)],
                                qhs[:, :, ds(256 * c, 256)],
                                start=True,
                                stop=True,
                                perf_mode=PM.DoubleRow,
                            )
                        e = ep.tile([P, SQ], BF, tag="e", name=f"e{h}_{t}")
                        nc.scalar.activation(e[:], ps[:], AF.Exp, scale=2.0 ** -9)
                        nc.vector.copy_predicated(e[:], msk_sb[:, t, :], ones16[:])
                        for c2 in range(2):
                            nc.tensor.matmul(
                                pso[:, ds(512 * c2, 512)],
                                vaug[:, t, ds(65 * h, 65)],
                                e[:, ds(512 * c2, 512)],
                                start=(t == 0),
                                stop=(t == 15),
                            )
                    def make_norm(h, pso):
                        def emit():
                            tot = totp.tile([65, SQ], F32, tag="tot", name=f"tot{h}")
                            nc.vector.tensor_copy(tot[:], pso[:])
                            rcp = rbp.tile([1, SQ], FR, tag="rcp", name=f"rcp{h}")
                            with nc.allow_low_precision(reason="f32r recip, multiplicative use"):
                                nc.vector.reciprocal(rcp[:], tot[64:65, :])
                            rb = rbps.tile([64, SQ], F32, tag="rb", name=f"rb{h}")
                            for _cb in range(2):
                                nc.tensor.matmul(
                                    rb[:, ds(512 * _cb, 512)],
                                    onesr[:, 0:64],
                                    rcp[:, ds(512 * _cb, 512)],
                                    start=True,
                                    stop=True,
                                )
                            nc.vector.scalar_tensor_tensor(
                                ao_sb[ds(64 * (h % 2), 64), h // 2, :],
                                tot[0:64, :],
                                1.0,
                                rb[:],
                                MULT,
                                MULT,
                            )
                        return emit
                    pending[0] = make_norm(h, pso)
                pending[0]()

            # ---------------- output projection ----------------
            if 'oproj' not in phases:
                nc.compile()
                return nc
            with (
                tc.tile_pool(name="pp2", bufs=4, space="PSUM") as pp2,
                tc.tile_pool(name="ost", bufs=3) as ost,
            ):
                for s in range(8):
                    for c in range(2):
                        ps = pp2.tile([P, 512], F32, tag="op", name=f"po{s}_{c}")
                        for g in range(8):
                            nc.tensor.matmul(
                                ps[:],
                                ao_sb[:, g, ts(s, P)],
                                wo_sb[:, g, ds(512 * c, 512)],
                                start=(g == 0),
                                stop=False,
                            )
                        nc.tensor.matmul(
                            ps[:],
                            ones16[0:1, 0:P],
                            bor_sb[:, ds(512 * c, 512)],
                            start=False,
                            stop=True,
                        )
                        oo = ost.tile([P, 512], F32, tag="oo")
                        nc.scalar.activation(oo[:], ps[:], AF.Copy, scale=1.0)
                        nc.sync.dma_start(out.ap()[ts(s, P), ds(512 * c, 512)], oo[:])

    nc.compile()
    return nc


def make_host_inputs(q, k, v, mask, Wq, bq, Wk, bk, Wv, bv, Wo, bo):
    """Full inputs -> list of 8 per-core input dicts."""
    q = np.asarray(q, np.float32)
    k = np.asarray(k, np.float32)
    v = np.asarray(v, np.float32)
    mask = np.asarray(mask)

    def f8(a):
        return np.ascontiguousarray(a, dtype=np.float32).astype(NF8)

    def bf(a):
        return np.ascontiguousarray(a, dtype=np.float32).astype(NBF)

    shared = {
        "wq8": f8(32.0 * np.asarray(Wq, np.float32).transpose(1, 0, 2).reshape(D, HK)),
        "wk8": f8(32.0 * np.asarray(Wk, np.float32).transpose(1, 0, 2).reshape(D, HK)),
        "wv16": bf(np.asarray(Wv, np.float32).transpose(1, 0, 2).reshape(D, HK)),
        "wo16": bf(np.asarray(Wo, np.float32)),
        "bqc": np.ascontiguousarray(
            8.0 * np.asarray(bq, np.float32).reshape(HK).reshape(8, P).T
        ),
        "bkc": np.ascontiguousarray(
            8.0 * np.asarray(bk, np.float32).reshape(HK).reshape(8, P).T
        ),
        "bvr": bf(np.asarray(bv, np.float32).reshape(1, HK)),
        "bor": bf(np.asarray(bo, np.float32).reshape(1, D)),
    }

    in_maps = []
    for core in range(8):
        b, j = divmod(core, 2)
        qs = q[b, j * SQ : (j + 1) * SQ, :]
        ms = mask[b, j * SQ : (j + 1) * SQ, :]
        m = dict(shared)
        m["qT8"] = f8(qs.T)
        m["kT8"] = f8(k[b].T)
        m["vT16"] = bf(v[b].T)
        m["mskT"] = np.ascontiguousarray((~ms).T).astype(np.uint16)
        in_maps.append(m)
    return in_maps


def assemble_output(results):
    """8 per-core out [SQ, D] -> full [4, 2048, 1024]."""
    B, S = 4, 2048
    full = np.empty((B, S, D), np.float32)
    for core, res in enumerate(results):
        b, j = divmod(core, 2)
        full[b, j * SQ : (j + 1) * SQ, :] = res["out"]
    return full


class CompiledSpmd:
    def __init__(self, nc: bass.Bass, n_cores: int):
        bass2jax.install_neuronx_cc_hook()
        assert nc.dbg_addr is None, "build with debug=False"
        partition_name = (
            nc.partition_id_tensor.name if nc.partition_id_tensor else None
        )
        in_names, out_names, out_avals, zero_outs = [], [], [], []
        for alloc in nc.m.functions[0].allocations:
            if not isinstance(alloc, mybir.MemoryLocationSet):
                continue
            name = alloc.memorylocations[0].name
            if alloc.kind == "ExternalInput":
                if name != partition_name:
                    in_names.append(name)
            elif alloc.kind == "ExternalOutput":
                shape = tuple(alloc.tensor_shape)
                dtype = mybir.dt.np(alloc.dtype)
                out_names.append(name)
                out_avals.append(jax.core.ShapedArray(shape, dtype))
                zero_outs.append(np.zeros(shape, dtype))
        n_params = len(in_names)
        n_outs = len(out_avals)
        all_in_names = list(in_names) + list(out_names)
        if partition_name is not None:
            all_in_names.append(partition_name)

        def _body(*args):
            operands = list(args)
            if partition_name is not None:
                operands.append(bass2jax.partition_id_tensor())
            outs = bass2jax._bass_exec_p.bind(
                *operands,
                out_avals=tuple(out_avals),
                in_names=tuple(all_in_names),
                out_names=tuple(out_names),
                lowering_input_output_aliases=(),
                sim_require_finite=True,
                sim_require_nnan=True,
                nc=nc,
            )
            return tuple(outs)

        devices = jax.devices()[:n_cores]
        assert len(devices) == n_cores
        mesh = Mesh(np.asarray(devices), ("core",))
        self._mesh = mesh
        donate = tuple(range(n_params, n_params + n_outs))
        self._sharded = jax.jit(
            shard_map(
                _body,
                mesh=mesh,
                in_specs=(PartitionSpec("core"),) * (n_params + n_outs),
                out_specs=(PartitionSpec("core"),) * n_outs,
                check_rep=False,
            ),
            donate_argnums=donate,
            keep_unused=True,
        )
        self.in_names = in_names
        self.out_names = out_names
        self.out_avals = out_avals
        self.zero_outs = zero_outs
        self.n_cores = n_cores

    def _concat_inputs(self, in_maps):
        per_core = [[np.asarray(m[n]) for n in self.in_names] for m in in_maps]
        return [
            np.concatenate([per_core[c][i] for c in range(self.n_cores)], axis=0)
            for i in range(len(self.in_names))
        ]

    def run(self, in_maps, repeats: int = 1):
        """Returns (results_per_core, wall_times_s list of len repeats)."""
        from jax.sharding import NamedSharding

        mesh = self._mesh
        shard = NamedSharding(mesh, PartitionSpec("core"))
        concat_in = [
            jax.device_put(a, shard) for a in self._concat_inputs(in_maps)
        ]
        rep_zeros = [
            [
                jax.device_put(
                    np.zeros((self.n_cores * z.shape[0], *z.shape[1:]), z.dtype),
                    shard,
                )
                for z in self.zero_outs
            ]
            for _ in range(repeats)
        ]
        jax.block_until_ready(concat_in)
        jax.block_until_ready(rep_zeros)
        times = []
        out_arrs = None
        for r in range(repeats):
            t0 = time.perf_counter()
            out_arrs = self._sharded(*concat_in, *rep_zeros[r])
            jax.block_until_ready(out_arrs)
            times.append(time.perf_counter() - t0)
        results = [
            {
                name: np.asarray(out_arrs[i]).reshape(
                    self.n_cores, *self.out_avals[i].shape
                )[c]
                for i, name in enumerate(self.out_names)
            }
            for c in range(self.n_cores)
        ]
        return results, times


_COMPILED = None


def _get_compiled():
    global _COMPILED
    if _COMPILED is None:
        nc = build_mha()
        _COMPILED = CompiledSpmd(nc, 8)
    return _COMPILED


def kernel(**inputs) -> np.ndarray:
    comp = _get_compiled()
    in_maps = make_host_inputs(**inputs)
    results, _ = comp.run(in_maps, repeats=1)
    return assemble_output(results)
